# revision 30
# baseline (speedup 1.0000x reference)
"""Box-SDF (CAPUDF box boundary distance) Trainium2 Bass kernel — fp16.

For each 3-D point x (S = 0.4):
    a_i = |x_i|                         (ACT Abs)
    b_i = relu(a_i - S) = max(a_i,S)-S  (DVE fused 2-op tensor_scalar, 4x)
    mx  = max_i a_i                     (DVE max tree)
    u   = min(mx, S) - S                (<= 0; == -(inside distance))
    b0' = b_0 + u                       (disjoint support: exact)
    d   = sqrt(b0'^2 + b1^2 + b2^2)     (= sqrt(sum b_i^2 + u^2))

Placement (measured fp16 rates): ACT = abs + 3/4 of the b1 square +
sqrt; DVE = everything else (self-read tensor_tensor squares have no
fp16 penalty); PE sums the 3 planes via identity-matmul 512-col PSUM
chunks, consuming DVE-fed planes first so the ACT square can lag one
tile; sqrt+store run per PSUM half to shorten the drain.

Tiles: 4 x 2048 points/partition-row, 2-stage software pipeline;
the Tile scheduler + 4-deep engine wait queues handle overlap.

Device I/O fp16 (host converts; L2 rel err ~4e-4 vs 2e-2 gate).
Sharding: data-parallel over points across 8 NeuronCores.
"""

import sys

import numpy as np

sys.path.insert(0, "/opt/trn_rl_repo")

import concourse.bacc as bacc  # noqa: E402
import concourse.mybir as mybir  # noqa: E402
from concourse import bass_utils  # noqa: E402
from concourse.tile import TileContext  # noqa: E402

N = 8388608
NCORES = 8
NPC = N // NCORES  # 1,048,576 points per core
P = 128
KPL = NPC // P  # 8192 points per partition lane
K = 2048  # max points per partition row per tile
F3 = 3 * K
TILES = [2048, 2048, 2048, 1024, 1024]  # last tile split: shorter drain
TOFF = [sum(TILES[:i]) for i in range(len(TILES))]
NT = len(TILES)

SIZE = 0.4
F16 = mybir.dt.float16
F32 = mybir.dt.float32
AF = mybir.ActivationFunctionType
OP = mybir.AluOpType


def build_kernel():
    nc = bacc.Bacc(
        "TRN2",
        target_bir_lowering=False,
        debug=False,
        num_devices=NCORES,
    )
    x = nc.dram_tensor("x", [P, 3 * KPL], F16, kind="ExternalInput").ap()
    eye = nc.dram_tensor("eye", [P, P], F16, kind="ExternalInput").ap()
    d = nc.dram_tensor("d", [P, KPL], F16, kind="ExternalOutput").ap()

    with TileContext(nc, pool_alloc_mode="queue") as tc:
        with (
            tc.tile_pool(name="const", bufs=1) as cpool,
            tc.tile_pool(name="xtp", bufs=3) as xtp,
            tc.tile_pool(name="big", bufs=2) as big,
            tc.tile_pool(name="bigb", bufs=3) as bigb,
            tc.tile_pool(name="small", bufs=3) as small,
            tc.tile_pool(name="psum", bufs=2, space="PSUM") as pspool,
        ):
            eye_t = cpool.tile([P, P], F16)
            state = {}

            def stage_a(t):
                k = TILES[t]
                xo = 3 * TOFF[t]
                xt = xtp.tile([P, F3], F16, tag="xt")
                aa = big.tile([P, F3], F16, tag="aa")
                if t == 0:
                    # chunk tile 0 per-plane so ACT starts sooner
                    for c in range(3):
                        cs = slice(c * k, (c + 1) * k)
                        nc.sync.dma_start(
                            out=xt[:, cs], in_=x[:, xo + c * k : xo + (c + 1) * k]
                        )
                        nc.scalar.activation(
                            out=aa[:, cs], in_=xt[:, cs], func=AF.Abs
                        )
                else:
                    nc.sync.dma_start(
                        out=xt[:, 0 : 3 * k], in_=x[:, xo : xo + 3 * k]
                    )
                    nc.scalar.activation(
                        out=aa[:, 0 : 3 * k], in_=xt[:, 0 : 3 * k], func=AF.Abs
                    )

                # b planes: relu(a - S) (fused max+add, 4x)
                bb = bigb.tile([P, F3], F16, tag="bb")
                nc.vector.tensor_scalar(
                    out=bb[:, 0 : 3 * k],
                    in0=aa[:, 0 : 3 * k],
                    scalar1=SIZE,
                    scalar2=-SIZE,
                    op0=OP.max,
                    op1=OP.add,
                )
                # mx tree
                m1 = small.tile([P, K], F16, tag="m1")
                nc.vector.tensor_tensor(
                    out=m1[:, 0:k], in0=aa[:, 0:k], in1=aa[:, k : 2 * k], op=OP.max
                )
                mx = small.tile([P, K], F16, tag="mx")
                nc.vector.tensor_tensor(
                    out=mx[:, 0:k], in0=m1[:, 0:k], in1=aa[:, 2 * k : 3 * k], op=OP.max
                )
                # u = min(mx,S)-S ; b0' = b0 + u
                u = small.tile([P, K], F16, tag="u")
                nc.vector.tensor_scalar(
                    out=u[:, 0:k],
                    in0=mx[:, 0:k],
                    scalar1=SIZE,
                    scalar2=-SIZE,
                    op0=OP.min,
                    op1=OP.add,
                )
                b0p = small.tile([P, K], F16, tag="b0p")
                nc.vector.tensor_tensor(
                    out=b0p[:, 0:k], in0=bb[:, 0:k], in1=u[:, 0:k], op=OP.add
                )

                # DVE self-squares: b0', b2 (b1 square runs on ACT)
                sq = bigb.tile([P, F3], F16, tag="sq")
                nc.vector.tensor_tensor(
                    out=sq[:, 0:k], in0=b0p[:, 0:k], in1=b0p[:, 0:k], op=OP.mult
                )
                nc.vector.tensor_tensor(
                    out=sq[:, 2 * k : 3 * k],
                    in0=bb[:, 2 * k : 3 * k],
                    in1=bb[:, 2 * k : 3 * k],
                    op=OP.mult,
                )
                state[t] = (bb, sq)

            def stage_b(t):
                k = TILES[t]
                bb, sq = state.pop(t)
                # ACT square: plane b1 (emitted here so abs(t+1) can slip ahead)
                nc.scalar.activation(
                    out=sq[:, k : 2 * k], in_=bb[:, k : 2 * k], func=AF.Square
                )
                s_ps = pspool.tile([P, K], F32, tag="s_ps")
                for j in range(0, k, 512):
                    for c in range(3):
                        nc.tensor.matmul(
                            s_ps[:, j : j + 512],
                            eye_t[:],
                            sq[:, c * k + j : c * k + j + 512],
                            start=(c == 0),
                            stop=(c == 2),
                        )
                dt = small.tile([P, K], F16, tag="dt")
                nc.scalar.activation(
                    out=dt[:, 0:k], in_=s_ps[:, 0:k], func=AF.Sqrt
                )
                nc.sync.dma_start(
                    out=d[:, TOFF[t] : TOFF[t] + k], in_=dt[:, 0:k]
                )

            stage_a(0)
            nc.sync.dma_start(out=eye_t[:], in_=eye[:])
            for t in range(1, NT):
                stage_a(t)
                stage_b(t - 1)
            stage_b(NT - 1)

    nc.compile()
    return nc


_cached_nc = None


def _get_nc():
    global _cached_nc
    if _cached_nc is None:
        _cached_nc = build_kernel()
    return _cached_nc


_AXON_SO = "/opt/axon/libaxon_pjrt.so"


def _ensure_ntff_hook():
    """Install an antenv.axon_hooks shim backed by libaxon_pjrt's NRT
    profiling C ABI, so run_bass_kernel_spmd(trace=True) works under axon."""
    try:
        from antenv.axon_hooks import get_axon_ntff_profile_hook  # noqa: F401

        return
    except ImportError:
        pass
    import contextlib
    import ctypes
    import types

    import antenv

    holder = {}
    mod = types.ModuleType("antenv.axon_hooks")
    mod.set_axon_ntff_profile_hook = lambda h: holder.__setitem__("h", h)
    mod.get_axon_ntff_profile_hook = lambda: holder.get("h")
    sys.modules["antenv.axon_hooks"] = mod
    antenv.axon_hooks = mod

    try:
        lib = ctypes.CDLL(_AXON_SO)
    except OSError:
        return
    if not hasattr(lib, "axon_start_nrt_profile"):
        return
    lib.axon_start_nrt_profile.argtypes = [
        ctypes.POINTER(ctypes.c_int64),
        ctypes.c_size_t,
    ]
    lib.axon_start_nrt_profile.restype = ctypes.c_int64
    lib.axon_stop_nrt_profile.argtypes = [ctypes.c_char_p]
    lib.axon_stop_nrt_profile.restype = ctypes.c_int64

    @contextlib.contextmanager
    def _hook(output_dir, device_ids):
        import jax

        jax.devices()
        if device_ids:
            ids = (ctypes.c_int64 * len(device_ids))(*device_ids)
            rc = lib.axon_start_nrt_profile(ids, len(device_ids))
        else:
            rc = lib.axon_start_nrt_profile(None, 0)
        if rc != 0:
            raise RuntimeError(f"axon_start_nrt_profile rc={rc}")
        try:
            yield
        finally:
            n = lib.axon_stop_nrt_profile(str(output_dir).encode())
            print(f"ntff profile: {n} file(s) written to {output_dir}")

    holder["h"] = _hook


def _host_shards(pts):
    """[N,3] f32 -> per-core [P, 3*KPL] fp16, planar per variable tile."""
    h = pts.astype(np.float16)
    g = h.reshape(NCORES, P, KPL, 3)
    out = np.empty((NCORES, P, 3 * KPL), dtype=np.float16)
    for k, off in zip(TILES, TOFF):
        blk = g[:, :, off : off + k, :].transpose(0, 1, 3, 2)
        out[:, :, 3 * off : 3 * (off + k)] = blk.reshape(NCORES, P, 3 * k)
    return out


def run(inputs_array, trace=False, **kwargs):
    """inputs_array: [N, 3] float32. Returns (out [N] float32, BassKernelResults)."""
    pts = np.ascontiguousarray(inputs_array, dtype=np.float32)
    assert pts.shape == (N, 3), pts.shape
    shards = _host_shards(pts)
    if trace:
        _ensure_ntff_hook()
    nc = _get_nc()
    eye_np = np.eye(P, dtype=np.float16)
    in_maps = [{"x": shards[i], "eye": eye_np} for i in range(NCORES)]
    res = bass_utils.run_bass_kernel_spmd(
        nc, in_maps, core_ids=list(range(NCORES)), trace=trace, **kwargs
    )
    out = np.concatenate(
        [res.results[i]["d"].reshape(-1) for i in range(NCORES)]
    ).astype(np.float32)
    return out, res


def kernel(**inputs):
    out, _ = run(inputs["inputs"])
    return out


if __name__ == "__main__":
    rng = np.random.default_rng(0)
    pts = rng.standard_normal((N, 3)).astype(np.float32)
    out, _ = run(pts)
    q = np.abs(pts) - SIZE
    inside = np.all(q < 0, axis=1)
    d_out = np.sqrt(np.sum(np.square(np.maximum(q, 0.0)), axis=1))
    d_in = -np.max(q, axis=1)
    exp = np.where(inside, d_in, d_out)
    err = np.abs(out - exp) / np.maximum(np.abs(exp), 1e-6)
    print("max rel err:", err.max(), "mean:", err.mean())
